# revision 26
# baseline (speedup 1.0000x reference)
"""AttnBlock (GroupNorm + single-head spatial self-attention + residual) on
8 Trainium2 NeuronCores.

Sharding: batch (4) x query-half (2) -> 8 independent shards, one per core.
Every core runs the SAME program on different data: the host rolls the
flattened spatial axis by 2048 for odd cores so each core's queries are the
first 2048 columns of its local x, while K/V see the full 4096.

The host precomputes all per-channel affine constants (GroupNorm mean/rstd
folded with gn affine into the conv weights, conv biases folded with the GN
shift, proj+v-bias folded into the residual) so the device does only the
dense work: Q/K/V 1x1 convs, scores, softmax, PV, and the residual add.

Device pipeline (per core), everything fp8e4 DoubleRow on the PE at
256-wide contraction (0.5 cycles/row):
  1. Q/K convs: DR matmuls vs pre-scaled fp8 weights; PSUM->SBUF cast with
     the folded bias on DVE emits q/k directly in fp8 [c_lo, c_hi, n].
  2. V conv emitted transposed [n, c] (lhsT = x tile) -> vT fp8, which is
     also pre-multiplied by the proj weight (wpv = wv^T wp^T), so the PV
     matmul directly produces the proj output.
  3. Attention per 512-query chunk: ST[j,i] = k^T q (DR), one ACT exp per
     256-key pair reads [128,1024] of PSUM and writes fp8 P directly
     (exp(s/16 - 3); the -3 keeps max P ~ 96 << 240 = fp8e4 max, softmax
     shift-invariance cancels it), PV accumulates [c,i] over key pairs.
     Softmax denominator Z via a tiny all-ones [128,2,1] DR stationary
     (out row [1,512]), reciprocal on that row + GpSimd partition
     broadcast, deferred one chunk so the cross-engine latency hides
     under the next chunk's matmul/exp stream.
  4. o = a * (1/Z) + (x + bias) (DVE), DMA out per chunk.
"""
import numpy as np
import ml_dtypes

B, C, H, W = 4, 256, 64, 64
N = H * W            # 4096 spatial positions
NQ = N // 2          # 2048 queries per core
P = 128              # partitions
CT = C // P          # 2 channel tiles
NUM_GROUPS = 8
EPS = 1e-5
SCALE = float(C) ** -0.5
EBIAS = -3.0         # exp(s/16 - 3): max scaled score ~7.6 -> max P ~ 96
IC_W = 512
NIC = NQ // IC_W     # 4 query chunks
NJP = N // 256       # 16 key pairs

F8 = ml_dtypes.float8_e4m3
BF16 = ml_dtypes.bfloat16

_CACHED = {}


def _build():
    import concourse.bass as bass
    import concourse.mybir as mybir
    import concourse.tile as tile
    from concourse import bacc

    dt = mybir.dt
    AF = mybir.ActivationFunctionType
    DR = mybir.MatmulPerfMode.DoubleRow

    nc = bacc.Bacc("TRN2", debug=False, num_devices=8)

    xf8_d = nc.dram_tensor("xf8", [P, CT * N], dt.float8e4, kind="ExternalInput")
    wall_d = nc.dram_tensor("wall", [P, 3 * CT * C], dt.float8e4,
                            kind="ExternalInput")
    aux_d = nc.dram_tensor("aux", [P, 8], dt.float32, kind="ExternalInput")
    xb_d = nc.dram_tensor("xb", [P, CT * NQ], dt.bfloat16, kind="ExternalInput")
    out_d = nc.dram_tensor("out", [C, NQ], dt.float32, kind="ExternalOutput")

    out_ap = out_d.ap().rearrange("(t p) n -> p t n", p=P)

    with tile.TileContext(nc) as tc:
        with (
            nc.allow_low_precision(reason="fp8 attention is intentional"),
            tc.tile_pool(name="persist", bufs=1) as pe_,
            tc.tile_pool(name="pt", bufs=2) as ptp,
            tc.tile_pool(name="tmp", bufs=4) as tmp,
            tc.tile_pool(name="st", bufs=2, space="PSUM") as stp,
            tc.tile_pool(name="acc", bufs=2, space="PSUM") as accp,
            tc.tile_pool(name="zp", bufs=1, space="PSUM") as zpp,
            tc.tile_pool(name="cv", bufs=1, space="PSUM") as cvp,
        ):
            # ---------- DMA in ----------
            # x is host-packed chunk-major [p, ck, t, 512] so the first conv
            # chunk is one contiguous transfer; weights land right after it,
            # the x bulk and the residual (needed late) follow
            # the first conv chunk gets its own tile so its readiness doesn't
            # depend (via coarse tile tracking) on the x bulk transfer
            xf8a = pe_.tile([P, 1, CT, 512], dt.float8e4, tag="xf8a")
            xf8b = pe_.tile([P, 7, CT, 512], dt.float8e4, tag="xf8b")
            wsc_all = pe_.tile([P, 3, CT, C], dt.float8e4, tag="wsc")
            aux_sb = pe_.tile([P, 8], dt.float32, tag="aux")
            with tc.high_priority():
                nc.sync.dma_start(
                    xf8a.rearrange("p a t n -> p (a t n)"),
                    xf8_d.ap()[:, 0:1024],
                )
                nc.sync.dma_start(
                    wsc_all.rearrange("p a t o -> p (a t o)"), wall_d.ap()
                )
                nc.sync.dma_start(aux_sb, aux_d.ap())
            nc.sync.dma_start(
                xf8b.rearrange("p c t n -> p (c t n)"),
                xf8_d.ap()[:, 1024 : CT * N],
            )

            def xf8(ck):
                return xf8a[:, 0] if ck == 0 else xf8b[:, ck - 1]
            wsc = {"q": wsc_all[:, 0], "k": wsc_all[:, 1], "v": wsc_all[:, 2]}
            bfq = aux_sb[:, 0:2]
            bfk = aux_sb[:, 2:4]
            xb = pe_.tile([P, CT, NQ], dt.bfloat16, tag="xb")
            nc.gpsimd.dma_start(xb.rearrange("p t n -> p (t n)"), xb_d.ap())

            ones_t = pe_.tile([P, 2, 16], dt.float8e4, tag="ones")
            nc.vector.memset(ones_t, 1.0)
            ones = ones_t[:, :, 0:1]
            ebias = pe_.tile([P, 1], dt.float32, tag="ebias")
            nc.vector.memset(ebias, EBIAS)

            k_sb = pe_.tile([P, CT, N], dt.float8e4, tag="k")
            q_sb = pe_.tile([P, CT, NQ], dt.float8e4, tag="q")
            vT = pe_.tile([P, NJP, 2, C], dt.float8e4, tag="vT")

            # ---------- conv emitters (called interleaved with attention) ----
            def conv_qk(nm, dst, bias, ck, on_act=False):
                # conv psum lives in the acc pool: a-tiles aren't allocated
                # until chunk 1, so convs get their own double-buffer and
                # never block the st rotation. The first chunk's casts go on
                # ACT (idle before the exp stream starts) so the four
                # first-ST input casts don't serialize on DVE.
                for h in range(CT):
                    cp = accp.tile([P, 512], dt.float32, tag="acc",
                                   name=f"c{nm}{h}_{ck}")
                    nc.tensor.matmul(
                        cp,
                        wsc[nm][:, :, h * P : (h + 1) * P],
                        xf8(ck),
                        start=True, stop=True, perf_mode=DR,
                    )
                    dsl = dst[:, h, ck * 512 : (ck + 1) * 512]
                    if on_act:
                        nc.scalar.activation(
                            dsl, cp, AF.Identity, bias=bias[:, h : h + 1],
                            scale=1.0,
                        )
                    else:
                        nc.vector.tensor_scalar_add(
                            dsl, cp, bias[:, h : h + 1]
                        )

            def conv_v(jp):
                # both key tiles of pair jp into one psum bank, one cast
                vp = cvp.tile([P, 2, C], dt.float32, tag="cv", name=f"cv{jp}")
                for u in range(2):
                    jt = 2 * jp + u
                    nc.tensor.matmul(
                        vp[:, u, :],
                        xf8(jt // 4)[:, :, (jt % 4) * P : (jt % 4 + 1) * P],
                        wsc["v"],
                        start=True, stop=True, perf_mode=DR,
                    )
                nc.vector.tensor_copy(vT[:, jp], vp)

            # ---------- attention ----------
            # PV/Z for chunk ic run one chunk late, interleaved into chunk
            # ic+1's ST/exp stream: chunk 0's PE slack absorbs the convs and
            # the softmax finalize latency always hides under live matmuls.
            pend = {}

            def emit_pv(ic, jp):
                a_ps, pts = pend[ic]["a"], pend[ic]["pts"]
                for ch in range(CT):
                    nc.tensor.matmul(
                        a_ps[ch],
                        vT[:, jp, :, ch * P : (ch + 1) * P],
                        pts[:, jp],
                        start=(jp == 0), stop=(jp == NJP - 1),
                        perf_mode=DR,
                    )

            # PV pacing: jp 0..11 one pair each, jp 12/13 two pairs each, so
            # the accumulation stops two blocks early and the o-epilogue
            # (which frees the a banks for the next chunk) overlaps the
            # remaining ST/exp stream instead of the chunk boundary
            PV_SCHED = [[jp] for jp in range(12)] + [[12, 13], [14, 15], [], []]

            def emit_z2(ic, zjp):
                pts, z_ps = pend[ic]["pts"], pend[ic]["z"]
                for jp in (zjp, zjp + 1):
                    nc.tensor.matmul(
                        z_ps, ones, pts[:, jp],
                        start=(jp == 0), stop=(jp == NJP - 1), perf_mode=DR,
                    )

            def emit_fin(ic):
                z_ps = pend[ic]["z"]
                zc = tmp.tile([1, 3, IC_W], dt.float32, tag="zc", name=f"zc{ic}")
                nc.vector.tensor_copy(zc[:, 0, :], z_ps)
                nc.vector.reciprocal_approx_accurate(
                    zc[:, 1, :], zc[:, 0, :], zc[:, 2, :]
                )
                zb = tmp.tile([P, IC_W], dt.float32, tag="zb", name=f"zb{ic}")
                nc.gpsimd.partition_broadcast(zb, zc[:, 1, :])
                pend[ic]["zb"] = zb

            def emit_out(ic):
                st_ = pend.pop(ic)
                isl, a_ps, zb = st_["isl"], st_["a"], st_["zb"]
                for ch in range(CT):
                    o = tmp.tile([P, IC_W], dt.float32, tag="o",
                                 name=f"o{ic}_{ch}")
                    nc.vector.tensor_mul(o, a_ps[ch], zb)
                    nc.vector.tensor_add(o, o, xb[:, ch, isl])
                    nc.sync.dma_start(out_ap[:, ch, isl], o)

            def deferred(ic, jp):
                # PV/Z/finalize work for chunk ic, paced by chunk ic+1's jps
                if ic < 0:
                    return
                if jp == 0:
                    pend[ic]["a"] = [
                        accp.tile([P, IC_W], dt.float32, tag="acc",
                                  name=f"a{ic}_{c}")
                        for c in range(CT)
                    ]
                    pend[ic]["z"] = zpp.tile([1, IC_W], dt.float32, tag="z",
                                             name=f"z{ic}")
                for pjp in PV_SCHED[jp]:
                    emit_pv(ic, pjp)
                if 1 <= jp <= 4:
                    # finish Z early so the reciprocal/broadcast chain
                    # overlaps live matmuls (matters most for the drain)
                    emit_z2(ic, 4 * (jp - 1))
                    emit_z2(ic, 4 * (jp - 1) + 2)
                if jp == 5:
                    emit_fin(ic)
                if jp == NJP - 1:
                    emit_out(ic)

            # prime: convs needed before chunk 0 can start
            conv_qk("k", k_sb, bfk, 0, on_act=True)
            conv_qk("q", q_sb, bfq, 0, on_act=True)

            for ic in range(NIC):
                isl = slice(ic * IC_W, (ic + 1) * IC_W)
                pts = ptp.tile([P, NJP, 2, IC_W], dt.float8e4, tag="pt",
                               name=f"pt{ic}")
                pend[ic] = {"isl": isl, "pts": pts}
                for jp in range(NJP):
                    # ready deferred work first so a blocked ST can't
                    # head-block it in the in-order PE queue
                    deferred(ic - 1, jp)
                    st = stp.tile([P, 2, IC_W], dt.float32, tag="st")
                    for u in range(2):
                        jt = 2 * jp + u
                        nc.tensor.matmul(
                            st[:, u, :],
                            k_sb[:, :, jt * P : (jt + 1) * P],
                            q_sb[:, :, isl],
                            start=True, stop=True, perf_mode=DR,
                        )
                    nc.scalar.activation(
                        pts[:, jp], st, AF.Exp, bias=ebias, scale=SCALE
                    )
                    # drip-feed conv work between blocks: k/q during chunk 0
                    # (k chunk c gates chunk 0's jp 2c-1), v during chunk 1
                    # (vT[jp] gates the deferred PV at (chunk 1, jp))
                    if ic == 0:
                        if jp % 2 == 1 and (jp + 1) // 2 < 8:
                            conv_qk("k", k_sb, bfk, (jp + 1) // 2)
                        if 2 <= jp <= 4:
                            conv_qk("q", q_sb, bfq, jp - 1)
                        if jp >= 14:
                            conv_v(jp - 14)
                    elif ic == 1 and jp < 14:
                        conv_v(jp + 2)
            # drain: deferred work for the last chunk
            for jp in range(NJP):
                deferred(NIC - 1, jp)

    nc.compile()
    return nc


def _get_nc():
    if "nc" not in _CACHED:
        _CACHED["nc"] = _build()
    return _CACHED["nc"]


def kernel(x, gn_scale, gn_bias, wq, bq, wk, bk, wv, bv, wp, bp,
           _trace=False, _trace_cores=None):
    try:
        import jax
        if jax.config.jax_compilation_cache_dir is None:
            jax.config.update("jax_compilation_cache_dir",
                              "/tmp/attnblock_jax_cache")
            jax.config.update("jax_persistent_cache_min_compile_time_secs", 1.0)
    except Exception:
        pass
    from concourse.bass_utils import run_bass_kernel_spmd

    nc = _get_nc()
    x = np.asarray(x, np.float64).reshape(B, C, N)
    gn_scale = np.asarray(gn_scale, np.float64)
    gn_bias = np.asarray(gn_bias, np.float64)
    wq64 = np.asarray(wq, np.float64)
    wk64 = np.asarray(wk, np.float64)
    wv64 = np.asarray(wv, np.float64)
    wp64 = np.asarray(wp, np.float64)
    bq64 = np.asarray(bq, np.float64)
    bk64 = np.asarray(bk, np.float64)
    bv64 = np.asarray(bv, np.float64)
    bp64 = np.asarray(bp, np.float64)

    # GroupNorm statistics per batch -> per-channel affine (host prep)
    g = NUM_GROUPS
    xg = x.reshape(B, g, C // g, N)
    mean = xg.mean(axis=(2, 3))                    # [B, g]
    var = xg.var(axis=(2, 3))
    rstd = 1.0 / np.sqrt(var + EPS)
    mean_c = np.repeat(mean, C // g, axis=1)       # [B, C]
    rstd_c = np.repeat(rstd, C // g, axis=1)
    alpha = rstd_c * gn_scale[None, :]             # [B, C]
    beta = gn_bias[None, :] - mean_c * alpha       # [B, C]

    wpv = wv64.T @ wp64.T                          # [c_in, o]
    bpbv = bp64 + wp64 @ bv64

    def pack8(a):  # [c, cols] f64 -> [128, 2*cols] fp8 (c = t*128 + p)
        a32 = np.clip(a, -240.0, 240.0).astype(np.float32)
        return np.ascontiguousarray(
            np.concatenate([a32[:P], a32[P:]], axis=1)
        ).astype(F8)

    def pack8_ck(a):  # [c, n] -> [128, (n/512) * 2 * 512] chunk-major fp8
        a32 = np.clip(a, -240.0, 240.0).astype(np.float32)
        st = np.stack([a32[:P], a32[P:]], axis=1)        # [128, 2, n]
        ck = st.reshape(P, 2, -1, 512).transpose(0, 2, 1, 3)
        return np.ascontiguousarray(ck.reshape(P, -1)).astype(F8)

    in_maps = []
    for core in range(8):
        b, qh = core // 2, core % 2
        xl = x[b]
        if qh == 1:
            xl = np.concatenate([xl[:, NQ:], xl[:, :NQ]], axis=1)
        # weights scaled by this batch's GN affine
        wscq = wq64.T * alpha[b][:, None]          # [c_in, o]
        wsck = wk64.T * alpha[b][:, None]
        wscv = wpv * alpha[b][:, None]
        bfq = bq64 + wq64 @ beta[b]
        bfk = bk64 + wk64 @ beta[b]
        bpp = bpbv + wpv.T @ beta[b]
        aux = np.zeros((P, 8), np.float32)
        aux[:, 0] = bfq[:P]; aux[:, 1] = bfq[P:]
        aux[:, 2] = bfk[:P]; aux[:, 3] = bfk[P:]
        xbl = (xl[:, :NQ] + bpp[:, None]).astype(np.float32)
        in_maps.append({
            "xf8": pack8_ck(xl),
            "wall": np.ascontiguousarray(np.concatenate(
                [pack8(wscq), pack8(wsck), pack8(wscv)], axis=1
            )),
            "aux": aux,
            "xb": np.ascontiguousarray(
                np.concatenate([xbl[:P], xbl[P:]], axis=1)
            ).astype(BF16),
        })

    last_err = None
    for attempt in range(3):
        try:
            res = run_bass_kernel_spmd(
                nc, in_maps, core_ids=list(range(8)), trace=_trace,
                trace_cores=_trace_cores,
            )
            break
        except Exception as e:  # transient NRT device faults happen rarely
            last_err = e
            import time as _time
            _time.sleep(2.0 * (attempt + 1))
    else:
        raise last_err
    out = np.empty((B, C, N), np.float32)
    for core in range(8):
        b, qh = core // 2, core % 2
        out[b][:, qh * NQ : (qh + 1) * NQ] = res.results[core]["out"]
    if _trace:
        _CACHED["last_results"] = res
    return out.reshape(B, C, H, W)
